# revision 15
# baseline (speedup 1.0000x reference)
"""GATv2ResidualNet Trainium2 kernel (8 NeuronCores, SPMD).

Strategy: destination-partitioned edge processing.
 - Nodes relabeled: global degree-sorted, round-robin dealt to 8 cores, so
   each core owns an equal node range and edge counts auto-balance; within a
   core, nodes are ordered by in-degree so fixed-shape destination blocks
   waste little padding.
 - Per layer: each core computes the att-folded source table (xl*att | el0)
   and the xr table for its own node slice, AllGathers the source table, then
   processes its destinations' incoming edges in 4 source-chunk passes
   (chunk width 25088 keeps dma_gather int16 indices in range).
 - Per-destination softmax is computed shift-free and single-pass:
   numer = sum(exp(e) * xl'[src]), denom = sum(exp(e)), out = numer/denom
   (identical math to the reference's max-shifted softmax; data scale makes
   overflow impossible). Per-dst partials accumulate in DRAM via
   dma_scatter_add across passes.
 - LeakyReLU/att contraction decomposition: lrelu(m)@att per head equals
   0.6*sum(z) + 0.4*sum(sign(att)*|z|) with z = m*att column-folded into the
   tables; the linear term's dst part cancels inside softmax and the src part
   rides in 8 extra table columns (el0). Epilogue divides by denom, unfolds
   att, adds bias/residual, applies exact gelu, and transposes h for the next
   layer's matmuls. Final linear runs fused into layer 4's epilogue.
"""
import sys
import numpy as np

sys.path.insert(0, '/opt/trn_rl_repo')

# ---------------- problem config (hardcoded) ----------------
N_NODES = 100_000
N_CORES = 8
PER = 12_544                    # nodes per core (incl. dummies) = 98*128
NPAD = N_CORES * PER            # 100352
NCHUNK = 4
CHW = NPAD // NCHUNK            # 25088 (< 32768: int16 gather idx fits)
NTILE = PER // 128              # 98
SCB = 8                         # blocks per scatter batch
SLOTCAP = 32                    # max padded slots per DVE sub-batch
CONST_W = 384

# per-layer: (fin, H, C, has_proj)
LAYERS = [(193, 8, 16, True), (128, 8, 16, False), (128, 8, 16, False), (128, 8, 8, True)]
HCs = [h * c for _, h, c, _ in LAYERS]
TDs = [128, 128, 128, 64]       # gather-table row width (f32): 512B stride keeps
                                # the Q7's 16-bit idx*stride_256 in range
NDs = [192, 192, 192, 128]      # numer scatter row width


def _ru(x, m):
    return (x + m - 1) // m * m


class Struct:
    pass


# ---------------- host-side graph prep ----------------
def prep_graph(edge_index):
    src = np.asarray(edge_index[0], np.int64)
    dst = np.asarray(edge_index[1], np.int64)
    deg = np.bincount(dst, minlength=N_NODES)
    order = np.argsort(-deg, kind='stable')
    new_of_old = np.empty(N_NODES, np.int64)
    ranks = np.arange(N_NODES)
    new_of_old[order] = (ranks % N_CORES) * PER + (ranks // N_CORES)

    srcN = new_of_old[src]
    dstN = new_of_old[dst]

    st = Struct()
    st.new_of_old = new_of_old

    core_of = dstN // PER
    dloc = dstN - core_of * PER
    chunk_of = srcN // CHW
    sloc = srcN - chunk_of * CHW

    # per (core, chunk): entries sorted by count desc: (count, local_dst, start)
    entries = [[None] * NCHUNK for _ in range(N_CORES)]
    for co in range(N_CORES):
        selc = core_of == co
        d_co, s_co, ch_co = dloc[selc], sloc[selc], chunk_of[selc]
        for ch in range(NCHUNK):
            m = ch_co == ch
            d_m, s_m = d_co[m], s_co[m]
            o = np.argsort(d_m, kind='stable')
            d_s, s_s = d_m[o], s_m[o]
            uq, starts, counts = np.unique(d_s, return_index=True, return_counts=True)
            o2 = np.argsort(-counts, kind='stable')
            entries[co][ch] = (counts[o2], uq[o2], starts[o2], s_s)

    # unified block structure
    st.nblk, st.blkW = [], []
    for ch in range(NCHUNK):
        nb = max(_ru(len(entries[co][ch][0]), 128) // 128 for co in range(N_CORES))
        nb = max(nb, 1)
        st.nblk.append(nb)
        Ws = []
        for b in range(nb):
            w = 1
            for co in range(N_CORES):
                cnts = entries[co][ch][0]
                if b * 128 < len(cnts):
                    w = max(w, int(cnts[b * 128]))
            Ws.append(w)
        st.blkW.append(Ws)

    # sub-batches: within each scatter group of SCB blocks, runs of blocks
    # processed by single wide DVE ops; W padded to the sub-batch max.
    # subs[(ch, bg)] = list of (k0, nsub, Wsb); gidx segments are per sub-batch.
    st.gidx_off, st.didx_off, st.subs = {}, {}, {}
    goff = doff = 0
    for ch in range(NCHUNK):
        for bg in range(0, st.nblk[ch], SCB):
            ngrp = min(SCB, st.nblk[ch] - bg)
            st.didx_off[(ch, bg)] = (doff, ngrp)
            doff += ngrp * 128
            subs = []
            k = 0
            while k < ngrp:
                Wsb = st.blkW[ch][bg + k]          # blocks sorted desc -> max
                nsub = 1
                while (k + nsub < ngrp
                       and (nsub + 1) * Wsb <= SLOTCAP):
                    nsub += 1
                subs.append((k, nsub, Wsb))
                st.gidx_off[(ch, bg, k)] = goff
                goff += 128 * nsub * Wsb
                k += nsub
            st.subs[(ch, bg)] = subs
    st.gidx_total, st.didx_total = goff, doff

    pad_row_local = CHW - 1
    st.gidx, st.didx = [], []
    for co in range(N_CORES):
        gflat = np.empty(goff, np.int16)
        dflat = np.empty(doff, np.int16)
        for ch in range(NCHUNK):
            cnts, uqs, starts, s_s = entries[co][ch]
            ne = len(cnts)
            nb = st.nblk[ch]
            for bg in range(0, nb, SCB):
                for (k0, nsub, Wsb) in st.subs[(ch, bg)]:
                    gi = np.full((nsub * Wsb, 128), pad_row_local, np.int64)
                    for k in range(nsub):
                        b = bg + k0 + k
                        for p in range(min(128, max(0, ne - b * 128))):
                            ei = b * 128 + p
                            c0, s0 = int(cnts[ei]), int(starts[ei])
                            gi[k * Wsb:k * Wsb + c0, p] = s_s[s0:s0 + c0]
                    flat = gi.reshape(-1)                          # i = slot*128+p
                    wrap = flat.reshape(-1, 16).T
                    o = st.gidx_off[(ch, bg, k0)]
                    gflat[o:o + flat.size] = wrap.reshape(-1).astype(np.int16)
            for bg in range(0, nb, SCB):
                o, ngrp = st.didx_off[(ch, bg)]
                di = np.full((ngrp, 128), PER - 1, np.int64)       # [k, p], i = k*128+p
                for k in range(ngrp):
                    lo = (bg + k) * 128
                    hi = min(lo + 128, ne)
                    if hi > lo:
                        di[k, :hi - lo] = uqs[lo:hi]
                flat = di.reshape(-1)
                wrap = flat.reshape(-1, 16).T
                dflat[o:o + ngrp * 128] = wrap.reshape(-1).astype(np.int16)
        st.gidx.append(gflat)
        st.didx.append(dflat)
    return st


# ---------------- per-layer host constants ----------------
def fold_layer(Wl, bl, Wr, br, att, bias):
    H, C = att.shape
    HC = H * C
    af = np.asarray(att, np.float32).reshape(HC)
    s = np.sign(af).astype(np.float32)
    s[s == 0] = 1.0
    Wlp = np.asarray(Wl, np.float32) * af
    blp = np.asarray(bl, np.float32) * af
    fl = Struct()
    fl.Wl_aug = Wlp.astype(np.float32)
    fl.bl_aug = blp.astype(np.float32)
    fl.Wrp = (np.asarray(Wr, np.float32) * af).astype(np.float32)
    fl.brp = (np.asarray(br, np.float32) * af).astype(np.float32)
    fl.s, fl.inv_att = s, (1.0 / af).astype(np.float32)
    fl.bias = np.asarray(bias, np.float32)
    fl.H, fl.C, fl.HC = H, C, HC
    return fl


def const_rows(fin):
    return fin + 8 + fin + 64     # weights | 8 small rows | proj | lin


def pack_consts(li, fl, proj_W, proj_b, lin_W, lin_b):
    fin, H, C, has_proj = LAYERS[li]
    HC = fl.HC
    buf = np.zeros((const_rows(fin), CONST_W), np.float32)
    buf[0:fin, 0:HC] = fl.Wl_aug
    buf[0:fin, CONST_W - HC:CONST_W] = fl.Wrp
    buf[fin, 0:HC] = fl.bl_aug
    buf[fin + 1, 0:HC] = fl.brp
    buf[fin + 2, 0:HC] = fl.s
    buf[fin + 3, 0:HC] = fl.inv_att
    buf[fin + 4, 0:HC] = fl.bias
    buf[fin + 5, 0:HC] = -20.0               # pad row: -20 in all xl' cols
    if has_proj:
        pw = np.asarray(proj_W).shape[1]
        buf[fin + 6, 0:pw] = np.asarray(proj_b, np.float32) + fl.bias
        buf[fin + 8:fin + 8 + fin, 0:pw] = np.asarray(proj_W, np.float32)
    if li == 3:
        buf[fin + 7, 0:32] = np.asarray(lin_b, np.float32)
        buf[fin + 8 + fin:fin + 8 + fin + 64, 0:32] = np.asarray(lin_W, np.float32)
    return buf


# ---------------- device program ----------------
def build_program(st):
    import dataclasses
    import concourse.bass as bass
    import concourse.mybir as mybir
    import concourse.tile as tile
    from concourse.bacc import Bacc
    from concourse.masks import make_identity

    f32 = mybir.dt.float32
    i16 = mybir.dt.int16

    nc = Bacc("TRN2", num_devices=N_CORES)

    xT = nc.dram_tensor("xT", [193, PER], f32, kind="ExternalInput")
    gidx_d = nc.dram_tensor("gidx", [_ru(st.gidx_total, 16)], i16, kind="ExternalInput")
    didx_d = nc.dram_tensor("didx", [_ru(st.didx_total, 16)], i16, kind="ExternalInput")
    consts_d = [nc.dram_tensor(f"consts{li}", [const_rows(LAYERS[li][0]), CONST_W], f32,
                               kind="ExternalInput") for li in range(4)]
    out_own = nc.dram_tensor("out_own", [PER, 32], f32, kind="ExternalOutput")

    tbl_own, tbl_full, xr_tab, num_tab, h_rows, hT_own, res_tab = [], [], [], [], [], [], []
    for li in range(4):
        TD, ND, HC = TDs[li], NDs[li], HCs[li]
        tbl_own.append(nc.dram_tensor(f"tblo{li}", [PER, TD], f32, kind="Internal"))
        tbl_full.append(nc.dram_tensor(f"tblf{li}", [NPAD, TD], f32, kind="Internal",
                                       addr_space="Shared"))
        xr_tab.append(nc.dram_tensor(f"xr{li}", [PER, HC], f32, kind="Internal"))
        num_tab.append(nc.dram_tensor(f"num{li}", [PER, ND], f32, kind="Internal"))
        h_rows.append(nc.dram_tensor(f"hr{li}", [PER, HC], f32, kind="Internal") if li < 3 else None)
        hT_own.append(nc.dram_tensor(f"hT{li}", [128, PER], f32, kind="Internal") if li < 3 else None)
        res_tab.append(nc.dram_tensor(f"res{li}", [PER, HC], f32, kind="Internal")
                       if LAYERS[li][3] else None)

    def rep_idx_ap(dram_t, off, n):
        """n idxs stored wrapped [16, n/16] C-order at element offset off ->
        [8(rep), 16, n/16] AP for a [128, n/16] SBUF tile."""
        cols = n // 16
        return bass.AP(dram_t.ap().tensor, off, [[0, 8], [cols, 16], [1, cols]])

    def row_bcast_ap(cst, row, width):
        return bass.AP(cst.ap().tensor, row * CONST_W, [[0, 128], [1, width]])

    def bc_mid(ap2, W):
        (ps, pc), (fs, fc) = ap2.ap[0], ap2.ap[1]
        return dataclasses.replace(ap2, ap=[[ps, pc], [0, W], [fs, fc]])

    def build_layer(li, hT_src, h_src_rows):
      with tile.TileContext(nc) as tc:
        with tc.tile_pool(name="consts", bufs=1) as cpool, \
             tc.tile_pool(name="node", bufs=3) as npool, \
             tc.tile_pool(name="psum", bufs=2, space="PSUM") as ppool, \
             tc.tile_pool(name="edge", bufs=2) as epool, \
             tc.tile_pool(name="scat", bufs=2) as spool:

            zero_t = cpool.tile([128, 192], f32)
            nc.vector.memset(zero_t[:], 0.0)
            ones_row = cpool.tile([1, 128], f32)
            nc.vector.memset(ones_row[:], 1.0)
            idt = cpool.tile([128, 128], f32)
            make_identity(nc, idt[:])

            if li == 3:
                cst3 = consts_d[3]
                lin_t = cpool.tile([64, 32], f32)
                nc.sync.dma_start(out=lin_t[:], in_=cst3[128 + 8 + 128:128 + 8 + 128 + 64, 0:32])
                linb_t = cpool.tile([1, 32], f32)
                nc.sync.dma_start(out=linb_t[:], in_=cst3[128 + 7:128 + 8, 0:32])

            prev_scatter = [None]

            if True:
                fin, H, C, has_proj = LAYERS[li]
                TD, ND, HC = TDs[li], NDs[li], HCs[li]
                cst = consts_d[li]
                k2 = fin - 128

                # ---- consts -> SBUF ----
                wl_t = cpool.tile([128, 2 * TD], f32, tag=f"wl{li}")
                nc.sync.dma_start(out=wl_t[:, 0:TD], in_=cst[0:128, 0:TD])
                if k2 > 0:
                    nc.sync.dma_start(out=wl_t[:k2, TD:2 * TD], in_=cst[128:fin, 0:TD])
                wr_t = cpool.tile([128, 2 * HC], f32, tag=f"wr{li}")
                nc.sync.dma_start(out=wr_t[:, 0:HC], in_=cst[0:128, CONST_W - HC:CONST_W])
                if k2 > 0:
                    nc.sync.dma_start(out=wr_t[:k2, HC:2 * HC],
                                      in_=cst[128:fin, CONST_W - HC:CONST_W])
                bl_t = cpool.tile([1, TD], f32, tag=f"bl{li}")
                nc.sync.dma_start(out=bl_t[:], in_=cst[fin:fin + 1, 0:TD])
                br_t = cpool.tile([1, HC], f32, tag=f"br{li}")
                nc.sync.dma_start(out=br_t[:], in_=cst[fin + 1:fin + 2, 0:HC])
                smat_t = cpool.tile([128, HC], f32, tag=f"sm{li}")
                nc.sync.dma_start(out=smat_t[:], in_=row_bcast_ap(cst, fin + 2, HC))
                invatt_t = cpool.tile([128, HC], f32, tag=f"ia{li}")
                nc.sync.dma_start(out=invatt_t[:], in_=row_bcast_ap(cst, fin + 3, HC))
                bias_t = cpool.tile([128, HC], f32, tag=f"bi{li}")
                nc.sync.dma_start(out=bias_t[:], in_=row_bcast_ap(cst, fin + 4, HC))
                pad_t = cpool.tile([1, TD], f32, tag=f"pr{li}")
                nc.sync.dma_start(out=pad_t[:], in_=cst[fin + 5:fin + 6, 0:TD])
                if has_proj:
                    pw = HC
                    pj_t = cpool.tile([128, 2 * pw], f32, tag=f"pj{li}")
                    nc.sync.dma_start(out=pj_t[:, 0:pw], in_=cst[fin + 8:fin + 8 + 128, 0:pw])
                    if k2 > 0:
                        nc.sync.dma_start(out=pj_t[:k2, pw:2 * pw],
                                          in_=cst[fin + 8 + 128:fin + 8 + fin, 0:pw])
                    pjb_t = cpool.tile([1, pw], f32, tag=f"pb{li}")
                    nc.sync.dma_start(out=pjb_t[:], in_=cst[fin + 6:fin + 7, 0:pw])

                # ---- node phase ----
                for t in range(NTILE):
                    sl = slice(t * 128, (t + 1) * 128)
                    lhs1 = npool.tile([128, 128], f32, tag="lhs1")
                    nc.sync.dma_start(out=lhs1[:], in_=hT_src[0:128, sl])
                    if k2 > 0:
                        lhs2 = npool.tile([128, 128], f32, tag="lhs2")
                        nc.sync.dma_start(out=lhs2[:k2], in_=hT_src[128:fin, sl])

                    ps = ppool.tile([128, TD], f32, tag="mm", space="PSUM")
                    nc.tensor.matmul(out=ps[:], lhsT=lhs1[:], rhs=wl_t[:, 0:TD],
                                     start=True, stop=False)
                    if k2 > 0:
                        nc.tensor.matmul(out=ps[:], lhsT=lhs2[:k2], rhs=wl_t[:k2, TD:2 * TD],
                                         start=False, stop=False)
                    nc.tensor.matmul(out=ps[:], lhsT=ones_row[:1, :],
                                     rhs=bl_t[:].to_broadcast([1, TD]) if False else bl_t[:],
                                     start=False, stop=True)
                    tabs = npool.tile([128, TD], f32, tag="tabs")
                    nc.scalar.copy(tabs[:], ps[:])
                    nc.sync.dma_start(out=tbl_own[li][sl], in_=tabs[:])

                    psr_f = ppool.tile([128, TD], f32, tag="mm", space="PSUM")
                    psr = psr_f[:, 0:HC]
                    nc.tensor.matmul(out=psr, lhsT=lhs1[:], rhs=wr_t[:, 0:HC],
                                     start=True, stop=False)
                    if k2 > 0:
                        nc.tensor.matmul(out=psr, lhsT=lhs2[:k2], rhs=wr_t[:k2, HC:2 * HC],
                                         start=False, stop=False)
                    nc.tensor.matmul(out=psr, lhsT=ones_row[:1, :], rhs=br_t[:],
                                     start=False, stop=True)
                    xrs = npool.tile([128, HC], f32, tag="xrs")
                    nc.scalar.copy(xrs[:], psr)
                    nc.sync.dma_start(out=xr_tab[li][sl], in_=xrs[:])

                    if has_proj:
                        psp_f = ppool.tile([128, TD], f32, tag="mm", space="PSUM")
                        psp = psp_f[:, 0:HC]
                        nc.tensor.matmul(out=psp, lhsT=lhs1[:], rhs=pj_t[:, 0:HC],
                                         start=True, stop=False)
                        if k2 > 0:
                            nc.tensor.matmul(out=psp, lhsT=lhs2[:k2], rhs=pj_t[:k2, HC:2 * HC],
                                             start=False, stop=False)
                        nc.tensor.matmul(out=psp, lhsT=ones_row[:1, :], rhs=pjb_t[:],
                                         start=False, stop=True)
                        rss = npool.tile([128, HC], f32, tag="rss")
                        nc.scalar.copy(rss[:], psp)
                        nc.sync.dma_start(out=res_tab[li][sl], in_=rss[:])

                    nc.sync.dma_start(out=num_tab[li][sl], in_=zero_t[:, 0:ND])

                nc.sync.dma_start(out=tbl_own[li][PER - 1:PER], in_=pad_t[:])

                # ---- AllGather ----
                nc.gpsimd.collective_compute(
                    kind="AllGather", op=mybir.AluOpType.bypass,
                    replica_groups=[list(range(N_CORES))],
                    ins=[tbl_own[li][:]], outs=[tbl_full[li][:]],
                )

                # ---- edge phase ----
                for ch in range(NCHUNK):
                    nb = st.nblk[ch]
                    for bg in range(0, nb, SCB):
                        doff, ngrp = st.didx_off[(ch, bg)]
                        nsc = ngrp * 128
                        dit = epool.tile([128, SCB * 8], i16, tag="dit")
                        nc.sync.dma_start(out=dit[:, 0:nsc // 16],
                                          in_=rep_idx_ap(didx_d, doff, nsc))
                        xrg = epool.tile([128, SCB, HC], f32, tag="xrg")
                        nc.gpsimd.dma_gather(xrg[:, 0:ngrp, :], xr_tab[li][:],
                                             dit[:, 0:nsc // 16], nsc, nsc, HC)
                        scb = spool.tile([128, SCB, ND], f32, tag="scb")
                        nc.vector.memset(scb[:, 0:ngrp, HC + 8:ND], 0.0)
                        for (k0, nsub, Wsb) in st.subs[(ch, bg)]:
                            NS = nsub * Wsb              # padded slots (s = k*Wsb+w)
                            goff = st.gidx_off[(ch, bg, k0)]
                            ngi = 128 * NS
                            git = epool.tile([128, _ru(SLOTCAP * 8, 2)], i16, tag="git")
                            nc.sync.dma_start(out=git[:, 0:ngi // 16],
                                              in_=rep_idx_ap(gidx_d, goff, ngi))
                            xlg = epool.tile([128, SLOTCAP * TD], f32, tag="xlg")
                            win_ap = bass.AP(tbl_full[li].ap().tensor, ch * CHW * TD,
                                             [[TD, CHW], [1, TD]])
                            for s0 in range(0, NS, 8):   # <=1024 idxs per gather
                                sn = min(8, NS - s0)
                                ni = sn * 128
                                nc.gpsimd.dma_gather(
                                    xlg[:, s0 * TD:(s0 + sn) * TD].rearrange(
                                        "p (w d) -> p w d", d=TD),
                                    win_ap, git[:, s0 * 8:s0 * 8 + ni // 16],
                                    ni, ni, TD)
                            pap = xlg[:].ap[0]
                            xo = xlg[:].offset

                            def rp(t, dims, off=0):
                                return dataclasses.replace(
                                    t[:], offset=t[:].offset + off,
                                    ap=[t[:].ap[0]] + dims)

                            # z = xl' + xr'  (s, f)
                            zt = epool.tile([128, SLOTCAP * HC], f32, tag="za")
                            nc.vector.tensor_tensor(
                                out=rp(zt, [[HC, NS], [1, HC]]),
                                in0=rp(xlg, [[TD, NS], [1, HC]]),
                                in1=rp(xrg, [[HC, nsub], [0, Wsb], [1, HC]], k0 * HC),
                                op=mybir.AluOpType.add)
                            # |z|
                            at = epool.tile([128, SLOTCAP * HC], f32, tag="at")
                            nc.scalar.activation(at[:, 0:NS * HC], zt[:, 0:NS * HC],
                                                 mybir.ActivationFunctionType.Abs)
                            # as = |z| * sign(att)   (s, f)
                            ast = epool.tile([128, SLOTCAP * HC], f32, tag="za")
                            nc.vector.tensor_tensor(
                                out=rp(ast, [[HC, NS], [1, HC]]),
                                in0=rp(at, [[HC, NS], [1, HC]]),
                                in1=rp(smat_t, [[0, NS], [1, HC]]),
                                op=mybir.AluOpType.mult)
                            # e1 = per-head sums (s, h)
                            e1 = epool.tile([128, SLOTCAP * 8], f32, tag="e1")
                            nc.vector.tensor_reduce(
                                e1[:, 0:NS * 8],
                                rp(ast, [[HC, NS], [C, H], [1, C]]),
                                axis=mybir.AxisListType.X, op=mybir.AluOpType.add)
                            # zsum = per-head sums of z (s, h); zr0 part cancels in softmax
                            zs = epool.tile([128, SLOTCAP * 8], f32, tag="zs")
                            nc.vector.tensor_reduce(
                                zs[:, 0:NS * 8],
                                rp(zt, [[HC, NS], [C, H], [1, C]]),
                                axis=mybir.AxisListType.X, op=mybir.AluOpType.add)
                            zs2 = epool.tile([128, SLOTCAP * 8], f32, tag="zs2")
                            nc.vector.tensor_scalar_mul(zs2[:, 0:NS * 8], zs[:, 0:NS * 8], 1.5)
                            # epre = e1 + 1.5*zsum   (s, h)
                            ep = epool.tile([128, SLOTCAP * 8], f32, tag="ep")
                            nc.vector.tensor_tensor(
                                out=rp(ep, [[8, NS], [1, 8]]),
                                in0=rp(e1, [[8, NS], [1, 8]]),
                                in1=rp(zs2, [[8, NS], [1, 8]]),
                                op=mybir.AluOpType.add)
                            # p = exp(0.4*epre), layout (h, s)
                            pb = epool.tile([128, SLOTCAP * 8], f32, tag="pb")
                            nc.scalar.activation(
                                rp(pb, [[1, NS], [NS, H]]),
                                rp(ep, [[8, NS], [1, 8]]),
                                mybir.ActivationFunctionType.Exp, scale=0.4)
                            # wtd = p (bcast C) * xl', layout (hc, s)
                            wb = epool.tile([128, SLOTCAP * HC], f32, tag="wb")
                            nc.vector.tensor_tensor(
                                out=rp(wb, [[1, NS], [C * NS, H], [NS, C]]),
                                in0=rp(xlg, [[TD, NS], [C, H], [1, C]]),
                                in1=rp(pb, [[1, NS], [NS, H], [0, C]]),
                                op=mybir.AluOpType.mult)
                            # numer -> scb[:, k0+k, 0:HC]  (k, hc) over w
                            nc.vector.tensor_reduce(
                                rp(scb, [[ND, nsub], [1, HC]], k0 * ND),
                                rp(wb, [[Wsb, nsub], [NS, HC], [1, Wsb]]),
                                axis=mybir.AxisListType.X, op=mybir.AluOpType.add)
                            # denom -> scb[:, k0+k, HC:HC+8]
                            nc.vector.tensor_reduce(
                                rp(scb, [[ND, nsub], [1, 8]], k0 * ND + HC),
                                rp(pb, [[Wsb, nsub], [NS, 8], [1, Wsb]]),
                                axis=mybir.AxisListType.X, op=mybir.AluOpType.add)
                        sc_i = nc.gpsimd.dma_scatter_add(
                            num_tab[li][:], scb[:, 0:ngrp, :], dit[:, 0:nsc // 16],
                            nsc, nsc, ND)
                        if prev_scatter[0] is not None:
                            tile.add_dep_helper(sc_i.ins, prev_scatter[0].ins, sync=True)
                        prev_scatter[0] = sc_i

                # ---- epilogue ----
                for t in range(NTILE):
                    sl = slice(t * 128, (t + 1) * 128)
                    nt = npool.tile([128, ND], f32, tag="nt")
                    nc.sync.dma_start(out=nt[:], in_=num_tab[li][sl])
                    den = npool.tile([128, 8], f32, tag="den")
                    nc.vector.tensor_scalar_add(den[:], nt[:, HC:HC + 8], 1e-30)
                    rden = npool.tile([128, 8], f32, tag="rden")
                    nc.vector.reciprocal(rden[:], den[:])
                    g1 = npool.tile([128, HC], f32, tag="g1")
                    nc.vector.tensor_tensor(
                        out=g1[:].rearrange("p (h c) -> p h c", c=C),
                        in0=nt[:, 0:HC].rearrange("p (h c) -> p h c", c=C),
                        in1=rden[:].to_broadcast([128, H, C]), op=mybir.AluOpType.mult)
                    g2 = npool.tile([128, HC], f32, tag="g2")
                    nc.vector.tensor_tensor(out=g2[:], in0=g1[:], in1=invatt_t[:],
                                            op=mybir.AluOpType.mult)
                    rs = npool.tile([128, HC], f32, tag="rs")
                    if has_proj:
                        nc.sync.dma_start(out=rs[:], in_=res_tab[li][sl])
                    else:
                        nc.sync.dma_start(out=rs[:], in_=h_src_rows[sl])
                    g3 = npool.tile([128, HC], f32, tag="g3")
                    nc.vector.tensor_tensor(out=g3[:], in0=g2[:], in1=rs[:],
                                            op=mybir.AluOpType.add)
                    if not has_proj:
                        g4 = npool.tile([128, HC], f32, tag="g4")
                        nc.vector.tensor_tensor(out=g4[:], in0=g3[:], in1=bias_t[:],
                                                op=mybir.AluOpType.add)
                    else:
                        g4 = g3
                    hh = npool.tile([128, HC], f32, tag="hh")
                    nc.scalar.activation(hh[:], g4[:], mybir.ActivationFunctionType.Gelu)

                    if li < 3:
                        nc.sync.dma_start(out=h_rows[li][sl], in_=hh[:])
                        tp = ppool.tile([128, 128], f32, tag="tp", space="PSUM")
                        nc.tensor.transpose(out=tp[:], in_=hh[:], identity=idt[:])
                        ht = npool.tile([128, 128], f32, tag="ht")
                        nc.scalar.copy(ht[:], tp[:])
                        nc.sync.dma_start(out=hT_own[li][:, sl], in_=ht[:])
                    else:
                        tp4_f = ppool.tile([128, 128], f32, tag="tp", space="PSUM")
                        tp4 = tp4_f[:64, :]
                        nc.tensor.transpose(out=tp4, in_=hh[:, 0:64], identity=idt[:])
                        h4t = npool.tile([64, 128], f32, tag="h4t")
                        nc.scalar.copy(h4t[:], tp4)
                        fo = ppool.tile([128, 32], f32, tag="fo", space="PSUM")
                        nc.tensor.matmul(out=fo[:], lhsT=h4t[:], rhs=lin_t[:],
                                         start=True, stop=False)
                        nc.tensor.matmul(out=fo[:], lhsT=ones_row[:1, :], rhs=linb_t[:],
                                         start=False, stop=True)
                        oo = npool.tile([128, 32], f32, tag="oo")
                        nc.scalar.copy(oo[:], fo[:])
                        nc.sync.dma_start(out=out_own[sl], in_=oo[:])

    build_layer(0, xT, None)
    build_layer(1, hT_own[0], h_rows[0])
    build_layer(2, hT_own[1], h_rows[1])
    build_layer(3, hT_own[2], h_rows[2])

    nc.compile()
    return nc


# ---------------- entry point ----------------
TRACE = False
LAST_EXEC_NS = None


def kernel(x, edge_index,
           Wl1, bl1, Wr1, br1, att1, bias1,
           Wl2, bl2, Wr2, br2, att2, bias2,
           Wl3, bl3, Wr3, br3, att3, bias3,
           Wl4, bl4, Wr4, br4, att4, bias4,
           proj1_W, proj1_b, proj4_W, proj4_b, lin1_W, lin1_b):
    from concourse.bass_utils import run_bass_kernel_spmd

    st = prep_graph(edge_index)
    nc = build_program(st)

    fls = [fold_layer(Wl1, bl1, Wr1, br1, att1, bias1),
           fold_layer(Wl2, bl2, Wr2, br2, att2, bias2),
           fold_layer(Wl3, bl3, Wr3, br3, att3, bias3),
           fold_layer(Wl4, bl4, Wr4, br4, att4, bias4)]
    projs = [(proj1_W, proj1_b), (None, None), (None, None), (proj4_W, proj4_b)]
    consts = [pack_consts(li, fls[li], projs[li][0], projs[li][1], lin1_W, lin1_b)
              for li in range(4)]

    xP = np.zeros((NPAD, 193), np.float32)
    xP[st.new_of_old] = np.asarray(x, np.float32)
    in_maps = []
    for co in range(N_CORES):
        xT_co = np.ascontiguousarray(xP[co * PER:(co + 1) * PER].T)
        gi = np.zeros(_ru(st.gidx_total, 16), np.int16)
        gi[:st.gidx_total] = st.gidx[co]
        di = np.zeros(_ru(st.didx_total, 16), np.int16)
        di[:st.didx_total] = st.didx[co]
        m = {"xT": xT_co, "gidx": gi, "didx": di}
        for li in range(4):
            m[f"consts{li}"] = consts[li]
        in_maps.append(m)

    global LAST_EXEC_NS
    import time as _time
    res = run_bass_kernel_spmd(nc, in_maps, core_ids=list(range(N_CORES)))
    t1 = _time.time()
    res = run_bass_kernel_spmd(nc, in_maps, core_ids=list(range(N_CORES)))
    LAST_EXEC_NS = int((_time.time() - t1) * 1e9)   # exec+transfer wall time

    outP = np.zeros((NPAD, 32), np.float32)
    for co in range(N_CORES):
        outP[co * PER:(co + 1) * PER] = res.results[co]["out_own"]
    return outP[st.new_of_old].astype(np.float32)
